# revision 1
# baseline (speedup 1.0000x reference)
"""Multi-head attention (fairseq-style, T-first) on 8 Trainium2 NeuronCores.

Sharding: 16 heads / 8 cores = 2 heads (128 channels) per core.
Each core computes its Q/K/V projections, attention for its 2 heads, and a
partial output projection attn_c @ Wo[:, c_slice].T  ->  [B*T, E].
The host sums the 8 partial outputs and adds bo.

Device-side layout notes:
  - host passes query pre-transposed, b-major:  queryT [E, B*T], col = b*T + t
  - q/k are produced transposed:  qT/kT [128 (2 heads x 64 d), B*T]
  - attention per (b, h) uses column-softmax:  scoresT [s, t] = kT_h.T @ qT_h
    exp without max-subtraction (|scores| <~ 7 for this problem's stats)
  - softmax denominator comes for free: V stationary is augmented with a
    ones column, so probs@V psum row 64 is sum(exp)
  - matmuls run as float32r (full PE rate at N>=256); probs/V in bf16
  - attention groups (b, tb) are software-pipelined: scores+exp of group i+1
    are emitted before probs@V of group i so the ACT engine (the bottleneck,
    ~147us of exp) never waits on PE program order
"""

import numpy as np

import concourse.bass as bass
import concourse.mybir as mybir
import concourse.tile as tile
from concourse import bacc
from concourse.bass_utils import run_bass_kernel_spmd
from concourse.masks import make_identity

T_, B_, E_, H_ = 2048, 2, 1024, 16
D_ = E_ // H_            # 64
R_ = B_ * T_             # 4096 tokens
NCORES = 8
EC = E_ // 128           # 8 contraction chunks for projections
PB = 512                 # projection token-block (matmul N)
TBLK = 1024              # attention t-block (exp instruction width)
SUB = 512                # pv / normalize sub-block (psum bank width)
SC = T_ // 128           # 16 s-chunks per batch
NG = R_ // TBLK          # 4 attention groups (b, tb)

f32 = mybir.dt.float32
f32r = mybir.dt.float32r
bf16 = mybir.dt.bfloat16
Exp = mybir.ActivationFunctionType.Exp


def _emit(nc, tc, dram):
    queryT, wq, wk, wv, wo, bq, bk, bv, outP = dram
    qview = queryT.ap().rearrange("(c p) r -> p c r", p=128)

    with tc.tile_pool(name="singles", bufs=1) as singles:
        wq_sb = singles.tile([128, EC, 128], bf16, name="wq_sb")
        wk_sb = singles.tile([128, EC, 128], bf16, name="wk_sb")
        wv_sb = singles.tile([128, EC, 128], bf16, name="wv_sb")
        wo_sb = singles.tile([128, E_], f32r, name="wo_sb")
        bq_sb = singles.tile([128, 1], f32, name="bq_sb")
        bk_sb = singles.tile([128, 1], f32, name="bk_sb")
        bv_sb = singles.tile([128, 1], f32, name="bv_sb")
        for w_dram, w_sb in ((wq, wq_sb), (wk, wk_sb), (wv, wv_sb)):
            nc.sync.dma_start(
                out=w_sb, in_=w_dram.ap().rearrange("(c p) j -> p c j", p=128)
            )
        nc.sync.dma_start(out=wo_sb, in_=wo.ap())
        nc.sync.dma_start(out=bq_sb, in_=bq.ap())
        nc.sync.dma_start(out=bk_sb, in_=bk.ap())
        nc.sync.dma_start(out=bv_sb, in_=bv.ap())

        id_sb = singles.tile([128, 128], f32, name="id_sb")
        make_identity(nc, id_sb)
        qT = singles.tile([128, R_], f32r, name="qT")
        kT = singles.tile([128, R_], f32r, name="kT")
        attn = singles.tile([128, R_], f32r, name="attn")
        # v in [s, d] layout per r'-chunk: [s, chunk, head, 64 v cols + ones]
        v_sb = singles.tile([128, R_ // 128, 2, 65], bf16, name="v_sb")
        nc.vector.memset(v_sb[:, :, :, 64], 1.0)

        # One pool scope for the whole kernel so projections (b1) can be
        # emitted between attention groups. PSUM: spool 2x[128,1024] = 4
        # banks, pvop 3x[128,512] = 3 banks (shared by proj psum, v-transpose,
        # probs@V and out-proj tiles), rbps 1 bank. Total = 8 banks.
        with (
            tc.tile_pool(name="xpool", bufs=2) as xpool,
            tc.tile_pool(name="vstage", bufs=2) as vstage,
            tc.tile_pool(name="spool", bufs=2, space="PSUM") as spool,
            tc.tile_pool(name="etpool", bufs=46) as etpool,
            tc.tile_pool(name="pvop", bufs=4, space="PSUM") as pvop,
            tc.tile_pool(name="rcpool", bufs=2) as rcpool,
            tc.tile_pool(name="hstage", bufs=2) as hstage,
            tc.tile_pool(name="opool", bufs=3) as opool,
        ):
            def proj_block(blk):
                cols = slice(blk * PB, (blk + 1) * PB)
                xb = xpool.tile([128, EC, PB], bf16, name="xb")
                # halves so the first e-chunk matmuls start after 1 MB, not 2
                half = EC // 2
                nc.sync.dma_start(out=xb[:, 0:half, :], in_=qview[:, 0:half, cols])
                nc.sync.dma_start(out=xb[:, half:, :], in_=qview[:, half:, cols])
                for w_sb, b_sb, dst in ((wq_sb, bq_sb, qT), (wk_sb, bk_sb, kT)):
                    ps = pvop.tile([128, PB], f32, name="ps", tag="pvop")
                    for ec in range(EC):
                        nc.tensor.matmul(
                            ps,
                            w_sb[:, ec, :],
                            xb[:, ec, :],
                            start=(ec == 0),
                            stop=(ec == EC - 1),
                        )
                    nc.vector.tensor_scalar_add(dst[:, cols], ps, b_sb)
                # v: same matmul, then transpose chunks into [s, d] layout
                ps = pvop.tile([128, PB], f32, name="ps", tag="pvop")
                for ec in range(EC):
                    nc.tensor.matmul(
                        ps,
                        wv_sb[:, ec, :],
                        xb[:, ec, :],
                        start=(ec == 0),
                        stop=(ec == EC - 1),
                    )
                vt = vstage.tile([128, PB], f32, name="vt")
                nc.vector.tensor_scalar_add(vt, ps, bv_sb)
                for i in range(PB // 128):
                    tp = pvop.tile([128, 128], f32, name="tp", tag="pvop")
                    nc.tensor.transpose(tp, vt[:, i * 128 : (i + 1) * 128], id_sb)
                    ch = blk * (PB // 128) + i
                    nc.vector.tensor_copy(
                        v_sb[:, ch, :, 0:64],
                        tp.rearrange("s (h d) -> s h d", h=2),
                    )

            ets = {}

            def scores_group(g):
                b, tb = divmod(g, T_ // TBLK)
                base = b * T_
                for sc_i in range(SC):
                    scols = slice(base + sc_i * 128, base + (sc_i + 1) * 128)
                    sps = []
                    for h in range(2):
                        sps.append(
                            spool.tile([128, TBLK], f32, name="sp", tag="sc")
                        )
                        ets[(g, h, sc_i)] = etpool.tile(
                            [128, TBLK], bf16, name="et", tag="et"
                        )
                    for s2 in range(TBLK // 512):
                        t2 = slice(
                            base + tb * TBLK + s2 * 512,
                            base + tb * TBLK + (s2 + 1) * 512,
                        )
                        for h in range(2):
                            dsl = slice(64 * h, 64 * (h + 1))
                            nc.tensor.matmul(
                                sps[h][:, s2 * 512 : (s2 + 1) * 512],
                                kT[dsl, scols],
                                qT[dsl, t2],
                                start=True,
                                stop=True,
                            )
                    for h in range(2):
                        nc.scalar.activation(ets[(g, h, sc_i)], sps[h], Exp)

            def pv_group(g):
                b, tb = divmod(g, T_ // TBLK)
                base = b * T_
                for h in range(2):
                    pvs = [
                        pvop.tile([128, SUB], f32, name="pv", tag="pvop")
                        for _ in range(TBLK // SUB)
                    ]
                    for sc_i in range(SC):
                        for sub in range(TBLK // SUB):
                            nc.tensor.matmul(
                                pvs[sub][0:65, :],
                                v_sb[:, b * SC + sc_i, h, :],
                                ets[(g, h, sc_i)][:, sub * SUB : (sub + 1) * SUB],
                                start=(sc_i == 0),
                                stop=(sc_i == SC - 1),
                            )
                    for sub in range(TBLK // SUB):
                        pv = pvs[sub]
                        rc = rcpool.tile([65, SUB], f32, name="rc", tag="rc")
                        nc.vector.reciprocal(rc[64:65, :], pv[64:65, :])
                        rbs = rcpool.tile([64, SUB], f32, name="rbs", tag="rbs")
                        srcap = rc[64:65, :]
                        bsrc = bass.AP(
                            tensor=srcap.tensor,
                            offset=srcap.offset,
                            ap=[list(srcap.ap[0]), [0, 64]] + list(srcap.ap[1:]),
                        )
                        nc.sync.dma_start(out=rbs, in_=bsrc)
                        ccols = slice(
                            base + tb * TBLK + sub * SUB,
                            base + tb * TBLK + (sub + 1) * SUB,
                        )
                        if h == 0:
                            nc.vector.tensor_mul(attn[0:64, ccols], pv[0:64, :], rbs)
                        else:
                            hs = hstage.tile([64, SUB], f32r, name="hs")
                            nc.vector.tensor_mul(hs, pv[0:64, :], rbs)
                            nc.sync.dma_start(out=attn[64:128, ccols], in_=hs)
                for h in range(2):
                    for sc_i in range(SC):
                        del ets[(g, h, sc_i)]

            def out_group(g):
                for i in range(TBLK // 128):
                    rch = g * (TBLK // 128) + i
                    rcols = slice(rch * 128, (rch + 1) * 128)
                    ot = opool.tile([128, E_ // 512, 512], f32, name="ot")
                    for jb in range(E_ // 512):
                        op = pvop.tile([128, 512], f32, name="op", tag="pvop")
                        nc.tensor.matmul(
                            op,
                            attn[:, rcols],
                            wo_sb[:, jb * 512 : (jb + 1) * 512],
                            start=True,
                            stop=True,
                        )
                        nc.vector.tensor_copy(ot[:, jb, :], op)
                    nc.sync.dma_start(out=outP.ap()[rcols, :], in_=ot)

            # b0 projections, then attention pipelined; b1 projections are
            # emitted under the first exp groups so PE stays ahead of ACT
            for blk in range(4):
                proj_block(blk)
            scores_group(0)
            scores_group(1)
            for blk in range(4, 8):
                proj_block(blk)
            pv_group(0)
            out_group(0)
            scores_group(2)
            pv_group(1)
            out_group(1)
            scores_group(3)
            pv_group(2)
            out_group(2)
            pv_group(3)
            out_group(3)


def build_kernel():
    nc = bacc.Bacc("TRN2", target_bir_lowering=False, debug=False)
    queryT = nc.dram_tensor("queryT", [E_, R_], bf16, kind="ExternalInput")
    wq = nc.dram_tensor("wqT", [E_, 128], bf16, kind="ExternalInput")
    wk = nc.dram_tensor("wkT", [E_, 128], bf16, kind="ExternalInput")
    wv = nc.dram_tensor("wvT", [E_, 128], bf16, kind="ExternalInput")
    wo = nc.dram_tensor("wo_cj", [128, E_], f32r, kind="ExternalInput")
    bq = nc.dram_tensor("bq", [128, 1], f32, kind="ExternalInput")
    bk = nc.dram_tensor("bk", [128, 1], f32, kind="ExternalInput")
    bv = nc.dram_tensor("bv", [128, 1], f32, kind="ExternalInput")
    outP = nc.dram_tensor("outP", [R_, E_], f32, kind="ExternalOutput")
    with tile.TileContext(nc) as tc:
        _emit(nc, tc, (queryT, wq, wk, wv, wo, bq, bk, bv, outP))
    nc.compile()
    return nc


_nc_cache = None


def get_nc():
    global _nc_cache
    if _nc_cache is None:
        _nc_cache = build_kernel()
    return _nc_cache


def prepare_in_maps(query, Wq, bq, Wk, bk, Wv, bv, Wo, bo):
    scaling = float(D_) ** -0.5
    query = np.asarray(query, np.float32)
    # [T, B, E] -> [E, B*T], column = b*T + t
    import ml_dtypes
    queryT = np.ascontiguousarray(
        query.transpose(2, 1, 0).reshape(E_, R_).astype(ml_dtypes.bfloat16)
    )
    in_maps = []
    for c in range(NCORES):
        sl = slice(128 * c, 128 * (c + 1))
        in_maps.append(
            {
                "queryT": queryT,
                "wqT": np.ascontiguousarray(
                    (np.asarray(Wq, np.float32)[sl] * scaling).T.astype(
                        ml_dtypes.bfloat16
                    )
                ),
                "wkT": np.ascontiguousarray(np.asarray(Wk, np.float32)[sl].T.astype(ml_dtypes.bfloat16)),
                "wvT": np.ascontiguousarray(np.asarray(Wv, np.float32)[sl].T.astype(ml_dtypes.bfloat16)),
                "wo_cj": np.ascontiguousarray(np.asarray(Wo, np.float32)[:, sl].T),
                "bq": np.ascontiguousarray(
                    (np.asarray(bq, np.float32)[sl] * scaling).reshape(128, 1)
                ),
                "bk": np.ascontiguousarray(np.asarray(bk, np.float32)[sl].reshape(128, 1)),
                "bv": np.ascontiguousarray(np.asarray(bv, np.float32)[sl].reshape(128, 1)),
            }
        )
    return in_maps


def finish_output(results, bo):
    total = np.zeros((R_, E_), np.float64)
    for r in results:
        total += r["outP"]
    out = total.astype(np.float32).reshape(B_, T_, E_).transpose(1, 0, 2)
    out = out + np.asarray(bo, np.float32)
    return np.ascontiguousarray(out.astype(np.float32))


def kernel(query, Wq, bq, Wk, bk, Wv, bv, Wo, bo, **_unused):
    nc = get_nc()
    in_maps = prepare_in_maps(query, Wq, bq, Wk, bk, Wv, bv, Wo, bo)
    res = run_bass_kernel_spmd(nc, in_maps, core_ids=list(range(NCORES)))
    return finish_output(res.results, bo)



# revision 5
# speedup vs baseline: 1.8243x; 1.8243x over previous
"""Multi-head attention (fairseq-style, T-first) on 8 Trainium2 NeuronCores.

Sharding: 16 heads / 8 cores = 2 heads (128 channels) per core.
Each core computes its Q/K/V projections, attention for its 2 heads, and a
partial output projection attn_c @ Wo[:, c_slice].T -> [B*T, E].  The 8
partials are summed ON DEVICE with a chunked ReduceScatter, bo is added on
device, and each core returns only its 2MB slice of the final output
([4, 128, E]: for group g it owns rows g*1024 + rank*128 .. +128).

I/O is fused into two input tensors per core (queryT + a weight blob) and
one small output, minimizing per-dispatch buffer traffic through the axon
relay.

Device-side layout notes:
  - host passes query pre-transposed, b-major:  queryT [E, B*T], col = b*T + t
  - q/k are produced transposed:  qT/kT [128 (2 heads x 64 d), B*T]
  - attention per (b, h) uses column-softmax:  scoresT [s, t] = kT_h.T @ qT_h
    exp without max-subtraction (|scores| <~ 7 for this problem's stats)
  - softmax denominator comes for free: V stationary is augmented with a
    ones column, so probs@V psum row 64 is sum(exp)
  - q/k matmuls run as float32r (full PE rate at N>=256); probs/V and the
    out-projection in bf16
  - attention groups (b, tb) are software-pipelined: scores+exp of group i+1
    are emitted before probs@V of group i so the ACT engine (the bottleneck,
    ~147us of exp) never waits on PE program order
  - after out-proj of group g, ReduceScatter(add) over cores 0-7 runs on
    that group's [1024, E] partial while later groups still compute
"""

import numpy as np

import concourse.bass as bass
import concourse.mybir as mybir
import concourse.tile as tile
from concourse import bacc
from concourse.bass_utils import run_bass_kernel_spmd
from concourse.masks import make_identity

T_, B_, E_, H_ = 2048, 2, 1024, 16
D_ = E_ // H_            # 64
R_ = B_ * T_             # 4096 tokens
NCORES = 8
EC = E_ // 128           # 8 contraction chunks for projections
PB = 512                 # projection token-block (matmul N)
TBLK = 1024              # attention t-block (exp instruction width)
SUB = 512                # pv / normalize sub-block (psum bank width)
SC = T_ // 128           # 16 s-chunks per batch
NG = R_ // TBLK          # 4 attention groups (b, tb)

# fused weight blob rows (all bf16, 128 cols):
#   [0:1024)      wqT  = (Wq[sl] * scaling).T          elem (e, ch)
#   [1024:2048)   wkT  = Wk[sl].T
#   [2048:3072)   wvT  = Wv[sl].T
#   [3072:4096)   wo   = Wo[:, sl].T flattened C-order, row k = flat[128k:128k+128]
#   [4096]        bq * scaling   [4097] bk   [4098] bv      (per-core slices)
#   [4099:4107)   bo (full, 8 rows of 128)
WF_ROWS = 4107

f32 = mybir.dt.float32
f32r = mybir.dt.float32r
bf16 = mybir.dt.bfloat16
Exp = mybir.ActivationFunctionType.Exp
GROUPS8 = [list(range(NCORES))]


def _emit(nc, tc, dram):
    queryT, wfuse, outS = dram
    qview = queryT.ap().rearrange("(c p) r -> p c r", p=128)
    wf = wfuse.ap()
    wq_view = wf[0:1024, :].rearrange("(c p) j -> p c j", p=128)
    wk_view = wf[1024:2048, :].rearrange("(c p) j -> p c j", p=128)
    wv_view = wf[2048:3072, :].rearrange("(c p) j -> p c j", p=128)
    wo_view = wf[3072:4096, :].rearrange("(p a) c -> p (a c)", p=128)

    with tc.tile_pool(name="singles", bufs=1) as singles:
        wq_sb = singles.tile([128, EC, 128], bf16, name="wq_sb")
        wk_sb = singles.tile([128, EC, 128], bf16, name="wk_sb")
        wv_sb = singles.tile([128, EC, 128], bf16, name="wv_sb")
        wo_sb = singles.tile([128, E_], bf16, name="wo_sb")
        for w_view, w_sb in ((wq_view, wq_sb), (wk_view, wk_sb), (wv_view, wv_sb)):
            nc.sync.dma_start(out=w_sb, in_=w_view)
        nc.sync.dma_start(out=wo_sb, in_=wo_view)

        # per-partition biases: blob row -> [128, 1]
        bq_bf = singles.tile([128, 3], bf16, name="bq_bf")
        for i in range(3):
            nc.sync.dma_start(
                out=bq_bf[:, i : i + 1],
                in_=wf[4096 + i : 4097 + i, :].rearrange("a c -> c a"),
            )
        b_f32 = singles.tile([128, 3], f32, name="b_f32")
        nc.vector.tensor_copy(b_f32, bq_bf)
        bq_sb = b_f32[:, 0:1]
        bk_sb = b_f32[:, 1:2]
        bv_sb = b_f32[:, 2:3]

        # bo broadcast to all partitions: [128, 1024] f32
        bo_src = wf[4099:4107, :]
        bo_bcast = bass.AP(
            tensor=bo_src.tensor,
            offset=bo_src.offset,
            ap=[[0, 128]] + list(bo_src.ap),
        )
        bo_bf = singles.tile([128, 8, 128], bf16, name="bo_bf")
        nc.sync.dma_start(out=bo_bf, in_=bo_bcast)
        bo_f = singles.tile([128, E_], f32, name="bo_f")
        nc.vector.tensor_copy(bo_f, bo_bf.rearrange("p a c -> p (a c)"))

        id_sb = singles.tile([128, 128], f32, name="id_sb")
        make_identity(nc, id_sb)
        qT = singles.tile([128, R_], f32r, name="qT")
        kT = singles.tile([128, R_], f32r, name="kT")
        attn = singles.tile([128, R_], bf16, name="attn")
        # v in [s, d] layout per r'-chunk: [s, chunk, head, 64 v cols + ones]
        v_sb = singles.tile([128, R_ // 128, 2, 65], bf16, name="v_sb")
        nc.vector.memset(v_sb[:, :, :, 64], 1.0)

        # One pool scope for the whole kernel so projections (b1) can be
        # emitted between attention groups. PSUM: spool 2x[128,1024] = 4
        # banks, pvop 3x[128,512] = 3 banks (shared by proj psum, v-transpose,
        # probs@V and out-proj tiles), rbps 1 bank. Total = 8 banks.
        with (
            tc.tile_pool(name="xpool", bufs=2) as xpool,
            tc.tile_pool(name="vstage", bufs=2) as vstage,
            tc.tile_pool(name="spool", bufs=2, space="PSUM") as spool,
            tc.tile_pool(name="etpool", bufs=46) as etpool,
            tc.tile_pool(name="pvop", bufs=4, space="PSUM") as pvop,
            tc.tile_pool(name="rcpool", bufs=2) as rcpool,
            tc.tile_pool(name="hstage", bufs=2) as hstage,
            tc.tile_pool(name="opool", bufs=3) as opool,
            tc.tile_pool(name="dpool", bufs=1, space="DRAM") as dpool,
        ):
            po = dpool.tile([NG, TBLK, E_], f32, name="po")
            rs_out = dpool.tile([NG, TBLK // NCORES, E_], f32, name="rs_out")

            def proj_block(blk):
                cols = slice(blk * PB, (blk + 1) * PB)
                xb = xpool.tile([128, EC, PB], bf16, name="xb")
                # halves so the first e-chunk matmuls start after 1 MB, not 2
                half = EC // 2
                nc.sync.dma_start(out=xb[:, 0:half, :], in_=qview[:, 0:half, cols])
                nc.sync.dma_start(out=xb[:, half:, :], in_=qview[:, half:, cols])
                for w_sb, b_sb, dst in ((wq_sb, bq_sb, qT), (wk_sb, bk_sb, kT)):
                    ps = pvop.tile([128, PB], f32, name="ps", tag="pvop")
                    for ec in range(EC):
                        nc.tensor.matmul(
                            ps,
                            w_sb[:, ec, :],
                            xb[:, ec, :],
                            start=(ec == 0),
                            stop=(ec == EC - 1),
                        )
                    nc.vector.tensor_scalar_add(dst[:, cols], ps, b_sb)
                # v: same matmul, then transpose chunks into [s, d] layout
                ps = pvop.tile([128, PB], f32, name="ps", tag="pvop")
                for ec in range(EC):
                    nc.tensor.matmul(
                        ps,
                        wv_sb[:, ec, :],
                        xb[:, ec, :],
                        start=(ec == 0),
                        stop=(ec == EC - 1),
                    )
                vt = vstage.tile([128, PB], f32, name="vt")
                nc.vector.tensor_scalar_add(vt, ps, bv_sb)
                for i in range(PB // 128):
                    tp = pvop.tile([128, 128], f32, name="tp", tag="pvop")
                    nc.tensor.transpose(tp, vt[:, i * 128 : (i + 1) * 128], id_sb)
                    ch = blk * (PB // 128) + i
                    nc.vector.tensor_copy(
                        v_sb[:, ch, :, 0:64],
                        tp.rearrange("s (h d) -> s h d", h=2),
                    )

            ets = {}

            def scores_group(g):
                b, tb = divmod(g, T_ // TBLK)
                base = b * T_
                for sc_i in range(SC):
                    scols = slice(base + sc_i * 128, base + (sc_i + 1) * 128)
                    sps = []
                    for h in range(2):
                        sps.append(
                            spool.tile([128, TBLK], f32, name="sp", tag="sc")
                        )
                        ets[(g, h, sc_i)] = etpool.tile(
                            [128, TBLK], bf16, name="et", tag="et"
                        )
                    for s2 in range(TBLK // 512):
                        t2 = slice(
                            base + tb * TBLK + s2 * 512,
                            base + tb * TBLK + (s2 + 1) * 512,
                        )
                        for h in range(2):
                            dsl = slice(64 * h, 64 * (h + 1))
                            nc.tensor.matmul(
                                sps[h][:, s2 * 512 : (s2 + 1) * 512],
                                kT[dsl, scols],
                                qT[dsl, t2],
                                start=True,
                                stop=True,
                            )
                    for h in range(2):
                        nc.scalar.activation(ets[(g, h, sc_i)], sps[h], Exp)

            def pv_group(g):
                b, tb = divmod(g, T_ // TBLK)
                base = b * T_
                for h in range(2):
                    pvs = [
                        pvop.tile([128, SUB], f32, name="pv", tag="pvop")
                        for _ in range(TBLK // SUB)
                    ]
                    for sc_i in range(SC):
                        for sub in range(TBLK // SUB):
                            nc.tensor.matmul(
                                pvs[sub][0:65, :],
                                v_sb[:, b * SC + sc_i, h, :],
                                ets[(g, h, sc_i)][:, sub * SUB : (sub + 1) * SUB],
                                start=(sc_i == 0),
                                stop=(sc_i == SC - 1),
                            )
                    for sub in range(TBLK // SUB):
                        pv = pvs[sub]
                        rc = rcpool.tile([65, SUB], f32, name="rc", tag="rc")
                        nc.vector.reciprocal(rc[64:65, :], pv[64:65, :])
                        rbs = rcpool.tile([64, SUB], f32, name="rbs", tag="rbs")
                        srcap = rc[64:65, :]
                        bsrc = bass.AP(
                            tensor=srcap.tensor,
                            offset=srcap.offset,
                            ap=[list(srcap.ap[0]), [0, 64]] + list(srcap.ap[1:]),
                        )
                        nc.sync.dma_start(out=rbs, in_=bsrc)
                        ccols = slice(
                            base + tb * TBLK + sub * SUB,
                            base + tb * TBLK + (sub + 1) * SUB,
                        )
                        if h == 0:
                            nc.vector.tensor_mul(attn[0:64, ccols], pv[0:64, :], rbs)
                        else:
                            hs = hstage.tile([64, SUB], bf16, name="hs")
                            nc.vector.tensor_mul(hs, pv[0:64, :], rbs)
                            nc.sync.dma_start(out=attn[64:128, ccols], in_=hs)
                for h in range(2):
                    for sc_i in range(SC):
                        del ets[(g, h, sc_i)]

            def out_group(g):
                for i in range(TBLK // 128):
                    rch = g * (TBLK // 128) + i
                    rcols = slice(rch * 128, (rch + 1) * 128)
                    ot = opool.tile([128, E_ // 512, 512], f32, name="ot", tag="ot")
                    for jb in range(E_ // 512):
                        op = pvop.tile([128, 512], f32, name="op", tag="pvop")
                        nc.tensor.matmul(
                            op,
                            attn[:, rcols],
                            wo_sb[:, jb * 512 : (jb + 1) * 512],
                            start=True,
                            stop=True,
                        )
                        nc.vector.tensor_copy(ot[:, jb, :], op)
                    nc.sync.dma_start(
                        out=po[g, i * 128 : (i + 1) * 128, :], in_=ot
                    )

            def rs_group(g):
                nc.gpsimd.collective_compute(
                    "ReduceScatter",
                    mybir.AluOpType.add,
                    replica_groups=GROUPS8,
                    ins=[po[g].opt()],
                    outs=[rs_out[g].opt()],
                )
                osb = opool.tile([128, E_], f32, name="osb", tag="ot")
                nc.sync.dma_start(out=osb, in_=rs_out[g])
                ofin = opool.tile([128, E_], f32, name="ofin", tag="ot")
                nc.vector.tensor_add(ofin, osb, bo_f)
                nc.sync.dma_start(out=outS.ap()[g], in_=ofin)

            # b0 projections, then attention pipelined; b1 projections are
            # emitted under the first exp groups so PE stays ahead of ACT
            for blk in range(4):
                proj_block(blk)
            scores_group(0)
            scores_group(1)
            for blk in range(4, 8):
                proj_block(blk)
            pv_group(0)
            out_group(0)
            rs_group(0)
            scores_group(2)
            pv_group(1)
            out_group(1)
            rs_group(1)
            scores_group(3)
            pv_group(2)
            out_group(2)
            rs_group(2)
            pv_group(3)
            out_group(3)
            rs_group(3)


def build_kernel():
    nc = bacc.Bacc(
        "TRN2",
        target_bir_lowering=False,
        debug=False,
        num_devices=NCORES,
        enable_partition_id=False,
    )
    queryT = nc.dram_tensor("queryT", [E_, R_], bf16, kind="ExternalInput")
    wfuse = nc.dram_tensor("wfuse", [WF_ROWS, 128], bf16, kind="ExternalInput")
    outS = nc.dram_tensor(
        "outS", [NG, TBLK // NCORES, E_], f32, kind="ExternalOutput"
    )
    with tile.TileContext(nc) as tc:
        _emit(nc, tc, (queryT, wfuse, outS))
    nc.compile()
    return nc


_nc_cache = None


def get_nc():
    global _nc_cache
    if _nc_cache is None:
        _nc_cache = build_kernel()
    return _nc_cache


def prepare_in_maps(query, Wq, bq, Wk, bk, Wv, bv, Wo, bo):
    scaling = float(D_) ** -0.5
    query = np.asarray(query, np.float32)
    # [T, B, E] -> [E, B*T], column = b*T + t
    import ml_dtypes
    queryT = np.ascontiguousarray(
        query.transpose(2, 1, 0).reshape(E_, R_).astype(ml_dtypes.bfloat16)
    )
    Wq = np.asarray(Wq, np.float32)
    Wk = np.asarray(Wk, np.float32)
    Wv = np.asarray(Wv, np.float32)
    Wo = np.asarray(Wo, np.float32)
    bo_rows = np.asarray(bo, np.float32).reshape(8, 128)
    in_maps = []
    for c in range(NCORES):
        sl = slice(128 * c, 128 * (c + 1))
        wf = np.empty((WF_ROWS, 128), ml_dtypes.bfloat16)
        wf[0:1024] = (Wq[sl] * scaling).T.astype(ml_dtypes.bfloat16)
        wf[1024:2048] = Wk[sl].T.astype(ml_dtypes.bfloat16)
        wf[2048:3072] = Wv[sl].T.astype(ml_dtypes.bfloat16)
        wf[3072:4096] = (
            np.ascontiguousarray(Wo[:, sl].T).reshape(1024, 128)
            .astype(ml_dtypes.bfloat16)
        )
        wf[4096] = (np.asarray(bq, np.float32)[sl] * scaling).astype(
            ml_dtypes.bfloat16
        )
        wf[4097] = np.asarray(bk, np.float32)[sl].astype(ml_dtypes.bfloat16)
        wf[4098] = np.asarray(bv, np.float32)[sl].astype(ml_dtypes.bfloat16)
        wf[4099:4107] = bo_rows.astype(ml_dtypes.bfloat16)
        in_maps.append({"queryT": queryT, "wfuse": wf})
    return in_maps


def finish_output(results, bo):
    # outS[c][g, i, :] = final output row g*1024 + c*128 + i (b-major tokens)
    stacked = np.stack([np.asarray(r["outS"]) for r in results])  # [8,4,128,E]
    full = stacked.transpose(1, 0, 2, 3).reshape(R_, E_)
    out = full.reshape(B_, T_, E_).transpose(1, 0, 2)
    return np.ascontiguousarray(out.astype(np.float32))


def kernel(query, Wq, bq, Wk, bk, Wv, bv, Wo, bo, **_unused):
    nc = get_nc()
    in_maps = prepare_in_maps(query, Wq, bq, Wk, bk, Wv, bv, Wo, bo)
    res = run_bass_kernel_spmd(nc, in_maps, core_ids=list(range(NCORES)))
    return finish_output(res.results, bo)
